# revision 14
# baseline (speedup 1.0000x reference)
"""MoE LoRA adapter layer (top-2 routed, E=8 experts, R=16) on 8 TRN2 NeuronCores.

Strategy: data-parallel over batch B=32 -> 4 batches/core; router + LoRA
weights replicated (tiny). E*R = 128 = partition width, so the per-expert
LoRA down/up projections stack into two dense matmuls over er=128:
    P1T[er, t] = D_all[er, :] @ x[t, :]^T          (contract H=1024)
    wT[h, t]   = U_all[:, h]^T @ (gate * P1T)      (contract er=128)
The expert sum IS the matmul contraction; gates (exactly 0 off the top-2)
fold in as a per-partition scale on P1T (tokens of one chunk share a batch).

x is shipped PRE-TRANSPOSED from host as xT[h, t] bf16 (host-side layout
prep, like the bf16 cast), so the kernel needs NO PE transposes: xT tiles
feed MM1 as rhs directly and MM2 emits wT[h, t] straight into bf16 PSUM.
The kernel stores wT only; the host adds the exact f32 x residual while
un-transposing (better precision than adding a bf16 x on device).

Every DMA moves one fully-contiguous 8 KiB run per partition (128
descriptors per transfer - HWDGE descriptor generation costs ~5 ns/desc
on the issuing sequencer). Loads ride the SP HWDGE ring, stores ride the
ACT HWDGE ring. PSUM-exit copies are split between DVE (2x bf16 mode) and
ACT. Gates are computed on-device in f32 (exact top-2) from a f32 cls copy.
"""

import sys

if "/opt/trn_rl_repo" not in sys.path:
    sys.path.insert(0, "/opt/trn_rl_repo")

import numpy as np
import ml_dtypes

import concourse.bass as bass
import concourse.tile as tile
from concourse import bacc, mybir
from concourse.bass_utils import run_bass_kernel_spmd

B, L, H = 32, 512, 1024
E, R, TOP_K = 8, 16, 2
N_CORES = 8
NB = B // N_CORES          # batches per core = 4
T = NB * L                 # tokens per core = 2048
P = 128                    # partitions
NK = H // P                # h-tiles = 8
NC = 4                     # chunks per core (1 chunk = 1 batch = 512 tokens)
TC = 512                   # tokens per chunk

F32 = mybir.dt.float32
BF16 = mybir.dt.bfloat16
BF16_NP = ml_dtypes.bfloat16

_COMPILED = None


def _build():
    """Build + compile the single-core program (same on all 8 cores)."""
    nc = bacc.Bacc("TRN2", target_bir_lowering=False, debug=False)

    # xT chunk-major: row (c, p) holds h-tiles k=0..7 for tokens of chunk c:
    # flat free index k*512 + t  <->  h = k*128 + p. 8 KiB contiguous rows.
    xt_in = nc.dram_tensor("xt_in", [NC * P, NK * TC], BF16, kind="ExternalInput")
    cls_in = nc.dram_tensor("cls_in", [NB, H], F32, kind="ExternalInput")
    wb_in = nc.dram_tensor("wb_in", [P, 2 * NK * P], BF16, kind="ExternalInput")
    cf_in = nc.dram_tensor("cf_in", [P, 320], F32, kind="ExternalInput")
    w_out = nc.dram_tensor("w_out", [NC * 2 * P, 4 * TC], BF16, kind="ExternalOutput")

    xt_v = xt_in.ap().rearrange("(c p) f -> c p f", c=NC, p=P)
    w_v = w_out.ap().rearrange("(c s p) f -> c s p f", c=NC, s=2, p=P)

    with tile.TileContext(nc) as tc:
        with (
            tc.tile_pool(name="wpool", bufs=1) as wpool,
            tc.tile_pool(name="xtpool", bufs=NC) as xtpool,
            tc.tile_pool(name="p2pool", bufs=2) as p2pool,
            tc.tile_pool(name="opool", bufs=2) as opool,
            tc.tile_pool(name="gpool", bufs=1) as gpool,
            tc.tile_pool(name="p1_ps", bufs=2, space="PSUM") as p1_ps,
            tc.tile_pool(name="w2_ps", bufs=6, space="PSUM") as w2_ps,
        ):
            # ---- all loads on the SP HWDGE ring, in consumption order ----
            cls_nat = gpool.tile([NB, H], F32, tag="cls")
            nc.sync.dma_start(cls_nat[:], cls_in.ap())
            cf_sb = wpool.tile([P, 320], F32, tag="cf")
            nc.sync.dma_start(cf_sb[:], cf_in.ap())
            wb_sb = wpool.tile([P, 2 * NK * P], BF16, tag="wb")

            id_sb = cf_sb[:, 0:128]
            rwt_sb = cf_sb[:, 128:192]       # [128, NK*E]
            rep_sb = cf_sb[0:E, 192:320]     # [8, 128]
            d_sb = wb_sb[:, 0 : NK * P]
            u_sb = wb_sb[:, NK * P : 2 * NK * P]

            # d before xt0 (MM1 dep), u after xt0 (only MM2 needs it)
            nc.sync.dma_start(d_sb, wb_in.ap()[:, 0 : NK * P])
            xt_tiles = []
            for c in range(NC):
                xt = xtpool.tile([P, NK * TC], BF16, tag="xt")
                nc.sync.dma_start(xt[:], xt_v[c])
                xt_tiles.append(xt)
                if c == 0:
                    nc.sync.dma_start(u_sb, wb_in.ap()[:, NK * P : 2 * NK * P])

            # ---- gates prologue (exact f32 top-2 softmax) -> gvec [128, NB] ----
            clsT = gpool.tile([P, NK * NB], F32, tag="clsT")
            cps = w2_ps.tile([P, TC], F32, tag="w")
            for k in range(NK):
                nc.tensor.transpose(
                    cps[:, k * NB : (k + 1) * NB],
                    cls_nat[:, k * P : (k + 1) * P],
                    id_sb[0:NB, 0:NB],
                )
            nc.vector.tensor_copy(clsT[:, 0 : NK * NB], cps[:, 0 : NK * NB])

            lg_ps = w2_ps.tile([P, TC], F32, tag="w")
            for k in range(NK):
                nc.tensor.matmul(
                    lg_ps[0:NB, 0:E],
                    clsT[:, k * NB : (k + 1) * NB],
                    rwt_sb[:, k * E : (k + 1) * E],
                    start=(k == 0),
                    stop=(k == NK - 1),
                )
            lg = gpool.tile([NB, E], F32, tag="lg")
            nc.vector.tensor_copy(lg[:], lg_ps[0:NB, 0:E])



            m1 = gpool.tile([NB, 1], F32, tag="m1")
            nc.vector.reduce_max(m1[:], lg[:], axis=mybir.AxisListType.X)
            t_sb = gpool.tile([NB, E], F32, tag="t")
            nc.vector.tensor_scalar(
                t_sb[:], lg[:], m1[:], None, op0=mybir.AluOpType.subtract
            )
            pen = gpool.tile([NB, E], F32, tag="pen")
            nc.vector.tensor_scalar(
                pen[:], t_sb[:], 0.0, 1e30,
                op0=mybir.AluOpType.is_ge, op1=mybir.AluOpType.mult,
            )
            t2 = gpool.tile([NB, E], F32, tag="t2")
            nc.vector.tensor_sub(t2[:], t_sb[:], pen[:])
            m2 = gpool.tile([NB, 1], F32, tag="m2")
            nc.vector.reduce_max(m2[:], t2[:], axis=mybir.AxisListType.X)
            keep = gpool.tile([NB, E], F32, tag="keep")
            nc.vector.tensor_scalar(
                keep[:], t_sb[:], m2[:], None, op0=mybir.AluOpType.is_ge
            )
            ex = gpool.tile([NB, E], F32, tag="ex")
            nc.scalar.activation(ex[:], t_sb[:], mybir.ActivationFunctionType.Exp)
            eg = gpool.tile([NB, E], F32, tag="eg")
            nc.vector.tensor_mul(eg[:], ex[:], keep[:])
            s_sb = gpool.tile([NB, 1], F32, tag="s")
            nc.vector.reduce_sum(s_sb[:], eg[:], axis=mybir.AxisListType.X)
            rs = gpool.tile([NB, 1], F32, tag="rs")
            nc.vector.reciprocal(rs[:], s_sb[:])
            gts = gpool.tile([NB, E], F32, tag="gts")
            nc.vector.tensor_scalar(
                gts[:], eg[:], rs[:], None, op0=mybir.AluOpType.mult
            )

            # ---- main loop: MM1 -> gate-scale -> MM2 -> copy out -> store ----
            p2_tiles = {}

            def stage_mm1(c):
                xt = xt_tiles[c]
                p1 = p1_ps.tile([P, TC], F32, tag="p1")
                for k in range(NK):
                    nc.tensor.matmul(
                        p1[:],
                        d_sb[:, k * P : (k + 1) * P],
                        xt[:, k * TC : (k + 1) * TC],
                        start=(k == 0),
                        stop=(k == NK - 1),
                    )
                p2_tiles[c] = p1

            def stage_scale(c):
                p1 = p2_tiles[c]
                p2t = p2pool.tile([P, TC], BF16, tag="p2t")
                nc.vector.tensor_scalar(
                    p2t[:], p1[:], gvec[:, c : c + 1], None,
                    op0=mybir.AluOpType.mult,
                )
                p2_tiles[c] = p2t

            def stage_mm2(c):
                p2t = p2_tiles[c]
                for s in range(2):
                    o_sb = opool.tile([P, 4 * TC], BF16, tag="o")
                    for j2 in range(4):
                        j = 4 * s + j2
                        wp = w2_ps.tile([P, TC], F32, tag="w")
                        nc.tensor.matmul(
                            wp[:],
                            u_sb[:, j * P : (j + 1) * P],
                            p2t[:],
                        )
                        dst = o_sb[:, j2 * TC : (j2 + 1) * TC]
                        if j % 2 == 0:
                            nc.vector.tensor_copy(dst, wp[:])
                        else:
                            nc.scalar.copy(dst, wp[:])
                    nc.sync.dma_start(w_v[c, s], o_sb[:])

            # Small bf16 PE warmup while xt0 finishes streaming (HAM unthrottle).
            junk = w2_ps.tile([P, TC], F32, tag="w")
            for _ in range(4):
                nc.tensor.matmul(
                    junk[:], d_sb[:, 0:P], d_sb[:, 0:TC], skip_group_check=True
                )

            # Software-pipelined emission: MM2(c) trails MM1(c+1) on the PE so
            # the DVE gate-scale round-trip latency is hidden. The gts->gvec PE
            # ops sit BEHIND MM1(0): they wait on the DVE softmax chain, which
            # completes while MM1(0) runs.
            stage_mm1(0)

            gt_ps = w2_ps.tile([P, TC], F32, tag="w")
            nc.tensor.transpose(gt_ps[0:E, 0:NB], gts[:], id_sb[0:NB, 0:NB])
            gtT = gpool.tile([E, NB], F32, tag="gtT")
            nc.vector.tensor_copy(gtT[:], gt_ps[0:E, 0:NB])
            gv_ps = w2_ps.tile([P, TC], F32, tag="w")
            nc.tensor.matmul(gv_ps[:, 0:NB], rep_sb[:], gtT[:])
            gvec = gpool.tile([P, NB], F32, tag="gvec")
            nc.vector.tensor_copy(gvec[:], gv_ps[:, 0:NB])

            stage_scale(0)
            stage_mm1(1)
            stage_scale(1)
            stage_mm2(0)
            stage_mm1(2)
            stage_scale(2)
            stage_mm2(1)
            stage_mm1(3)
            stage_scale(3)
            stage_mm2(2)
            stage_mm2(3)

    nc.compile()
    return nc


def _weights_maps(router_w, lora_down, lora_up):
    # D_all[(e,r), h] stacked; MM1 lhsT tiles need [p, k*128+m] = D_all[m, k*128+p]
    d_all = lora_down.reshape(E * R, H)                       # [128, 1024]
    d_t = d_all.T.reshape(NK, P, E * R).transpose(1, 0, 2).reshape(P, NK * P)
    # U_all[(e,r), h] = lora_up[e, h, r]; MM2 lhsT tile j = U_all[:, j*128:(j+1)*128]
    u_np = lora_up.transpose(0, 2, 1).reshape(E * R, H)       # [128, 1024]
    wb = np.ascontiguousarray(
        np.concatenate([d_t, u_np], axis=1)
    ).astype(BF16_NP)
    # router_wT tiles [p, k*E+e] = router_w[e, k*128+p]
    rwt_np = router_w.T.reshape(NK, P, E).transpose(1, 0, 2).reshape(P, NK * E)
    rep_np = np.zeros((P, P), np.float32)
    for e in range(E):
        rep_np[e, e * R : (e + 1) * R] = 1.0
    cf = np.ascontiguousarray(
        np.concatenate(
            [np.eye(P, dtype=np.float32), rwt_np.astype(np.float32), rep_np], axis=1
        )
    )
    return {"wb_in": wb, "cf_in": cf}


def get_compiled():
    global _COMPILED
    if _COMPILED is None:
        _COMPILED = _build()
    return _COMPILED


def make_in_maps(x, router_w, lora_down, lora_up):
    x = np.asarray(x, np.float32)
    w_maps = _weights_maps(
        np.asarray(router_w, np.float32),
        np.asarray(lora_down, np.float32),
        np.asarray(lora_up, np.float32),
    )
    in_maps = []
    for i in range(N_CORES):
        xc = x[i * NB : (i + 1) * NB]                         # [4, 512, 1024]
        # (c, t, h) -> (c, p, k, t) with h = k*128 + p, rows 8 KiB contiguous
        xt = np.ascontiguousarray(
            xc.transpose(0, 2, 1).reshape(NC, NK, P, TC).transpose(0, 2, 1, 3)
        ).astype(BF16_NP).reshape(NC * P, NK * TC)
        cls_shard = np.ascontiguousarray(xc[:, 0, :])
        in_maps.append({"xt_in": xt, "cls_in": cls_shard, **w_maps})
    return in_maps


def assemble_y(x_core, w_raw):
    """Per-core: y = x + unscramble(wT).  x_core f32 [NB, L, H]."""
    wt = np.asarray(w_raw, np.float32).reshape(NC, 2, P, 4, TC)
    # (c, s, p, j2, t) -> (c, t, s, j2, p) = [4, 512, 1024]
    w = wt.transpose(0, 4, 1, 3, 2).reshape(NB, L, H)
    return x_core + w


def kernel(x, router_w, lora_down, lora_up):
    nc = get_compiled()
    x = np.asarray(x, np.float32)
    in_maps = make_in_maps(x, router_w, lora_down, lora_up)
    res = run_bass_kernel_spmd(nc, in_maps, core_ids=list(range(N_CORES)))
    out = np.empty((B, L, H), np.float32)
    for i in range(N_CORES):
        out[i * NB : (i + 1) * NB] = assemble_y(
            x[i * NB : (i + 1) * NB], res.results[i]["w_out"]
        )
    return out
